# revision 3
# baseline (speedup 1.0000x reference)
"""Trainium2 Bass kernel for nn_AdaptiveThresholdNet_16930761080953.

Reference analysis (load-bearing):
  _volume_density() computes counts = sum(ones(idx.shape), axis=-1) — i.e. it
  sums ONES over the top-k axis, so counts == MAX_K (=64) for every point,
  independent of the xyz values.  The whole (B, N, N) cdist + top_k is dead
  code: dens is the constant MAX_K / (4/3*pi*r^3) everywhere, and
  d_mean = mean(dens, axis=1) is the same constant for every batch element
  (verified bitwise: perturbing xyz leaves the reference output unchanged).

  The live computation is therefore a 1->64->64->1 MLP evaluated once on the
  scalar d_mean, then broadcast to the batch:
      h1  = relu(d_mean * W1[:,0] + b1)            (64,)
      h2  = relu(W2 @ h1 + b2)                     (64,)
      t   = sigmoid(W3 @ h2 + b3)                  (1,)
      out = MIN_D + (MAX_D - MIN_D) * t  broadcast to (B,)

  d_mean is NOT exactly 64/vol in float32 — XLA's mean over 8192 identical
  values accumulates rounding.  The bit-exact constant (0x4174765f =
  15.278899) was extracted from the reference computation; using it makes the
  host-equivalent MLP reproduce the reference output bitwise.

Sharding: the live compute is ~500 FLOPs, so there is nothing to shard — the
tiny MLP is replicated on all 8 cores (SPMD) and core 0's output is taken.

Device layout: all weights are packed host-side into one (64, 70) f32 tensor
so the kernel needs a single input DMA:
  cols 0:64  -> W2.T   (contraction dim j on partitions, so PE's
                        lhsT.T @ rhs = W2 @ h1 with no on-device transpose)
  col  64    -> W1[:, 0]
  col  65    -> b1
  col  66    -> b2
  col  67    -> W3[0, :]  (as a column, for the final dot product on PE)
  [0]  68    -> b3[0]
  [0]  69    -> MIN_D (additive constant of the final affine)

Raw-bass engine plan (one input DMA covers every weight, so each cross-engine
hop needs exactly one semaphore wait):
  SP : dma_start(packed)          -> dsem += 16
       wait asem>=3; dma_start(out) -> dsem += 16; wait dsem>=32
  ACT: wait dsem>=16
       h1 = relu(dm*W1+b1)        -> asem += 1
       wait psem>=1; h2 = relu(z2+b2) -> asem += 1
       wait psem>=2; t = sigmoid(z3+b3) (bcast 1->B)
       thr = 40*t + 20            -> asem += 1
  PE : wait asem>=1; z2 = W2 @ h1 -> psem += 1
       wait asem>=2; z3 = h2 . w3 -> psem += 1
"""

import numpy as np

_N_CORES = 8
_B = 4  # batch size of this problem

# Bit-exact f32 of jnp.mean(full((8192,1), 64/vol)) from the reference.
_D_MEAN = float(np.frombuffer(bytes.fromhex("5f767441"), dtype="<f4")[0])
_MIN_D = 20.0
_SPAN_D = 40.0  # MAX_D - MIN_D

_CACHE = {}


def _build():
    from concourse import bass, mybir

    nc = bass.Bass()
    packed_p = nc.declare_dram_parameter(
        "packed", [64, 70], mybir.dt.float32, isOutput=False
    )
    out_p = nc.declare_dram_parameter("out", [1, _B], mybir.dt.float32, isOutput=True)

    AF = mybir.ActivationFunctionType
    f32 = mybir.dt.float32

    with (
        nc.sbuf_tensor("packed_sb", [64, 70], f32) as packed,
        nc.sbuf_tensor("h1", [64, 1], f32) as h1,
        nc.sbuf_tensor("h2", [64, 1], f32) as h2,
        nc.sbuf_tensor("t4", [1, _B], f32) as t4,
        nc.sbuf_tensor("thr", [1, _B], f32) as thr,
        nc.psum_tensor("z2", [64, 1], f32) as z2,
        nc.psum_tensor("z3", [1, 1], f32) as z3,
        nc.semaphore("dsem") as dsem,
        nc.semaphore("asem") as asem,
        nc.semaphore("psem") as psem,
        nc.Block() as block,
    ):

        @block.sync
        def _(sp):
            sp.dma_start(packed[:], packed_p[:]).then_inc(dsem, 16)
            sp.wait_ge(asem, 3)
            sp.dma_start(out_p[:], thr[:]).then_inc(dsem, 16)
            sp.wait_ge(dsem, 32)

        @block.scalar
        def _(act):
            act.wait_ge(dsem, 16)
            # h1 = relu(d_mean * W1 + b1)
            act.activation(
                h1[:], packed[:, 64:65], AF.Relu, bias=packed[:, 65:66], scale=_D_MEAN
            ).then_inc(asem, 1)
            act.wait_ge(psem, 1)
            # h2 = relu(z2 + b2)
            act.activation(h2[:], z2[:], AF.Relu, bias=packed[:, 66:67]).then_inc(
                asem, 1
            )
            act.wait_ge(psem, 2)
            # t = sigmoid(z3 + b3), broadcast (1,1) -> (1,B) via 0-stride AP
            z3b, t4b = bass.broadcast_tensor_aps(z3[:], t4[:])
            act.activation(t4b, z3b, AF.Sigmoid, bias=packed[0:1, 68:69])
            # ACT is pipelined: drain before the same-engine RAW read of t4
            act.drain()
            # thr = SPAN_D * t + MIN_D
            act.activation(
                thr[:], t4[:], AF.Identity, bias=packed[0:1, 69:70], scale=_SPAN_D
            ).then_inc(asem, 1)

        @block.tensor
        def _(pe):
            pe.wait_ge(asem, 1)
            # z2 = (W2T).T @ h1 = W2 @ h1
            pe.matmul(z2[:], packed[:, 0:64], h1[:], start=True, stop=True).then_inc(
                psem, 1
            )
            pe.wait_ge(asem, 2)
            # z3 = h2 . w3col
            pe.matmul(z3[:], h2[:], packed[:, 67:68], start=True, stop=True).then_inc(
                psem, 1
            )

    return nc


def _pack(inputs):
    W1 = np.asarray(inputs["W1"], dtype=np.float32)
    b1 = np.asarray(inputs["b1"], dtype=np.float32)
    W2 = np.asarray(inputs["W2"], dtype=np.float32)
    b2 = np.asarray(inputs["b2"], dtype=np.float32)
    W3 = np.asarray(inputs["W3"], dtype=np.float32)
    b3 = np.asarray(inputs["b3"], dtype=np.float32)

    packed = np.zeros((64, 70), dtype=np.float32)
    packed[:, 0:64] = W2.T
    packed[:, 64] = W1[:, 0]
    packed[:, 65] = b1
    packed[:, 66] = b2
    packed[:, 67] = W3[0, :]
    packed[0, 68] = b3[0]
    packed[0, 69] = np.float32(_MIN_D)
    return packed


def _run(inputs, trace=False):
    from concourse.bass_utils import run_bass_kernel_spmd

    if "nc" not in _CACHE:
        _CACHE["nc"] = _build()
    nc = _CACHE["nc"]

    packed = _pack(inputs)
    in_maps = [{"packed": packed} for _ in range(_N_CORES)]
    res = run_bass_kernel_spmd(nc, in_maps, core_ids=list(range(_N_CORES)), trace=trace)
    out = np.asarray(res.results[0]["out"], dtype=np.float32).reshape(_B)
    return out, res.exec_time_ns


def kernel(**inputs) -> np.ndarray:
    out, _ = _run(inputs, trace=False)
    return out
